# revision 17
# baseline (speedup 1.0000x reference)
"""Sparse attention (template/search) Trainium2 Bass kernel, v2.

Problem: B=128, N=320 (T=64 template + S=256 search), C=768, H=12, d=64.
  x = concat(x1[:, :64], x2[:, 64:])
  qkv = x @ qkv_w.T ; per-head attention (template->template, search->all)
  out = attn @ proj_w.T + proj_b

Strategy: pure data parallel over batch, 16 batches per core on 8 cores.

v2 design notes (vs the fp32r v1 baseline):
  * All PE operands are bf16 (PSUM accumulation stays fp32).  bf16 enables
    fast weight load (FWL), halves SBUF read bandwidth, and removes the
    4x penalty fp32r pays on sub-256 free-dim matmuls.  Tolerance is
    2e-2; bf16 lands ~1e-3.
  * Scores are computed transposed (S^T[s, q]) per head pair with the
    array row-packed (tile_position) so one Exp covers both heads.
  * Attention output is TOKEN-major: O[q, d] per head via
    lhsT = exp-tile [s, q-chunk], rhs = v [s, 65] (64 v cols + ones
    column so row .. col 64 carries the softmax denominator).  The
    normalizer is then per-PARTITION: one tiny DVE reciprocal [q, 6] per
    half + one broadcast tensor_mul -- no [1, N] single-partition
    reciprocal (which costs ~8 cycles/elem on DVE), no gpsimd broadcast.
  * The projection needs feature-major attn; 18 dma_start_transpose
    calls (DMA XBAR, 2-byte dtype) do the transpose on the idle DMA
    engines instead of PE/Act.
  * proj bias is applied by the Act engine during the PSUM->SBUF copy
    (activation Identity with a per-partition bias AP), not by a matmul.
  * Emission is software-pipelined: batch b+1's QKV/V matmuls are
    interleaved into batch b's attention phase so the PE never idles
    long enough for the HAM clock gate to re-throttle it to 1.2 GHz.

PSUM budget (8 banks): scores 2 (one [128,1024] tile, heads at col 0 /
512 so row-packed concurrent matmuls never share a bank), attnV 2
([*,390] tiles, bufs=2), qkv/v/proj 3, spare 1.
"""

import numpy as np

import concourse.bass as bass
import concourse.bacc as bacc
import concourse.mybir as mybir
from concourse.tile import TileContext
from concourse.bass_utils import run_bass_kernel_spmd

f32 = mybir.dt.float32
bf16 = mybir.dt.bfloat16
i32 = mybir.dt.int32
Exp = mybir.ActivationFunctionType.Exp
Identity = mybir.ActivationFunctionType.Identity

B, N, C = 128, 320, 768
H, D = 12, 64
T, S = 64, 256
N_CORES = 8
BPC = B // N_CORES  # batches per core

NCT = C // 128            # 6 c-tiles of 128
NQK = 2 * C // 128        # 12 qk row-tiles
NPAIR = H // 2            # 6 head pairs
S_TILES = [(0, 128), (128, 128), (256, 64)]   # (s0, ssz) key-token tiles
# output-token chunks: template 0:64, search 64:192, 192:320.
# per chunk: (q0, qsz, ex-col-offset); template chunk uses the mt scores
# (ex cols 256:320 of s-tile 2), search chunks use ex cols 0:128/128:256.
Q_CHUNKS = [(0, 64, None), (64, 128, 0), (192, 128, 128)]
SCALE = D ** -0.5
VW = 65                   # per-head V block width (64 v cols + ones)
HH = 6                    # heads per half


def build_bass(bpc: int = BPC, n_cores: int = N_CORES, reps: int = 1):
    nc = bacc.Bacc("TRN2", target_bir_lowering=False, debug=False,
                   num_devices=n_cores)

    # all inputs host-tiled so every DMA reads long contiguous runs:
    # xt[b, p, ct, t] = x^T; w*[mt, p, ct, mm] = w[ct*128+p, mt*128+mm]
    xt_d = nc.declare_dram_parameter("xt", [bpc, 128, NCT, N], bf16,
                                     isOutput=False)
    wqk_d = nc.declare_dram_parameter("wqk", [NQK, 128, NCT, 128], bf16,
                                      isOutput=False)
    wv_d = nc.declare_dram_parameter("wv", [NCT, 128, NCT, 128], bf16,
                                     isOutput=False)
    wp_d = nc.declare_dram_parameter("wp", [NCT, 128, NCT, 128], bf16,
                                     isOutput=False)
    pb_d = nc.declare_dram_parameter("pb", [NCT, 128], f32, isOutput=False)
    r_d = None
    if reps == 0:   # timing harness: runtime iteration count
        r_d = nc.declare_dram_parameter("reps_in", [1, 1], i32, isOutput=False)
    y_d = nc.declare_dram_parameter("y", [bpc, C, N], f32, isOutput=True)

    with TileContext(nc) as tc:
        with (
            tc.tile_pool(name="wpool", bufs=1) as wpool,
            tc.tile_pool(name="xpool", bufs=2) as xpool,
            tc.tile_pool(name="qkpool", bufs=2) as qkpool,
            tc.tile_pool(name="vpool", bufs=2) as vpool,
            tc.tile_pool(name="epool", bufs=7) as epool,
            tc.tile_pool(name="opool", bufs=4) as opool,
            tc.tile_pool(name="rpool", bufs=3) as rpool,
            tc.tile_pool(name="tokpool", bufs=4) as tokpool,
            tc.tile_pool(name="atpool", bufs=2) as atpool,
            tc.tile_pool(name="ypool", bufs=3) as ypool,
            # PSUM bank budget (8 banks of 2KB/partition):
            #   sc [128,1024]f32 = 2 banks x bufs=2       -> 4
            #   O  [128,390]f32  = 1 bank  x bufs=1       -> 1
            #   pacc/vacc/yp     = 1 bank  x bufs=1 each  -> 3
            tc.tile_pool(name="psA", bufs=1, space="PSUM") as psA,
            tc.tile_pool(name="psB", bufs=2, space="PSUM") as psB,
            tc.tile_pool(name="psO", bufs=1, space="PSUM") as psO,
        ):
            # ---- x prefetch + persistent weights ----
            # xt(0)/xt(1) are split across the three DMA queues and issued
            # FIRST; weight slices follow in prologue-consumption order
            # (wqk m0, wv mt0-3 for v chunk 0, ...) so the PE starts real
            # work ~7us in instead of ~25us.
            xts = {}

            def load_x(b, split=False):
                xts[b] = xpool.tile([128, NCT, N], bf16, name="xt")
                if split:
                    for i, eng in enumerate((nc.sync, nc.scalar, nc.gpsimd)):
                        eng.dma_start(out=xts[b][:, 2 * i:2 * i + 2],
                                      in_=xt_d[b][:, 2 * i:2 * i + 2])
                else:
                    nc.sync.dma_start(out=xts[b], in_=xt_d[b])

            if reps == 1:
                load_x(0, split=True)
                if bpc > 1:
                    load_x(1, split=True)

            wqk_sb = wpool.tile([128, NQK, NCT, 128], bf16)   # lhsT for q,k
            wv_sb = wpool.tile([128, NCT, NCT, 128], bf16)    # rhs for v
            wp_sb = wpool.tile([128, NCT, NCT, 128], bf16)    # lhsT for proj
            wload = ([("qk", 0), ("v", 0), ("v", 1), ("v", 2), ("v", 3),
                      ("qk", 1), ("v", 4), ("v", 5)]
                     + [("qk", m) for m in range(2, NQK)]
                     + [("p", m) for m in range(NCT)])
            for i, (kind, m) in enumerate(wload):
                eng = (nc.scalar, nc.gpsimd)[i % 2]
                sb, dr = {"qk": (wqk_sb, wqk_d), "v": (wv_sb, wv_d),
                          "p": (wp_sb, wp_d)}[kind]
                eng.dma_start(out=sb[:, m], in_=dr[m])
            pb_sb = wpool.tile([128, NCT], f32)            # per-partition bias
            nc.gpsimd.dma_start(out=pb_sb[:], in_=pb_d.rearrange("m p -> p m"))
            ones_f = wpool.tile([128, 1], f32)
            nc.vector.memset(ones_f[:], 1.0)
            rv = None
            if reps == 0:
                r_sb = wpool.tile([1, 1], i32)
                nc.sync.dma_start(out=r_sb[:], in_=r_d[:])
                tmp = nc.alloc_registers("reps_regs")
                nc.regs_load(tmp, r_sb[0:1, 0:1])
                rv = nc.snap(tmp, donate=True, min_val=1, max_val=4096)

            def body(_iv=None):
                # per-batch state carried across the pipelined emission
                xt = xts     # b -> xt tile
                qk = {}      # b -> qk tile
                vt = {}      # b -> v tile
                ex = {}      # (b, pair) -> exp tile
                osb = {}     # (b, chunk, half) -> O sbuf tile
                tok = {}     # (b, chunk) -> normalized token-major attn
                att = {}     # b -> feature-major attn (proj rhs)

                def qkv_chunk(b, m, tag="pacc"):
                    # one of the 12 q/k feature-major row-tiles
                    if m == 0:
                        qk[b] = qkpool.tile([128, NQK, N], bf16, name="qk")
                    pacc = psA.tile([128, N], f32, tag=tag, name="pacc")
                    for ct in range(NCT):
                        nc.tensor.matmul(
                            pacc[:, 0:N],
                            wqk_sb[:, m, ct],
                            xt[b][:, ct, :],
                            start=(ct == 0), stop=(ct == NCT - 1))
                    nc.vector.tensor_copy(qk[b][:, m, :], pacc[:, 0:N])

                def v_chunk(b, c):
                    # token-major v with per-head ones column; 6 chunks:
                    # c = tt*2 + (0: heads 0-7 cols 0:512, 1: heads 8-11)
                    tt, ci = divmod(c, 2)
                    tsz = 64 if tt == 2 else 128
                    c0, csz = (0, 512) if ci == 0 else (512, 256)
                    if c == 0:
                        vt[b] = vpool.tile([128, 3, H * VW], bf16, name="vt")
                    pacc = psA.tile([128, 512], f32, tag="vacc",
                                    name="vacc")
                    for ct in range(NCT):
                        nc.tensor.matmul(
                            pacc[0:tsz, 0:csz],
                            xt[b][:, ct, tt * 128:tt * 128 + tsz],
                            bass.AP(tensor=wv_sb.tensor,
                                    offset=wv_sb.offset
                                    + (c0 // 128) * NCT * 128 + ct * 128,
                                    ap=[wv_sb.ap[0], [NCT * 128, csz // 128],
                                        [1, 128]]),
                            start=(ct == 0), stop=(ct == NCT - 1))
                    nh = csz // D
                    h0 = c0 // D
                    dst = vt[b][0:tsz, tt, h0 * VW:(h0 + nh) * VW] \
                        .rearrange("p (h c) -> p h c", c=VW)[:, :, 0:D]
                    src = pacc[0:tsz, 0:csz] \
                        .rearrange("p (h c) -> p h c", c=D)
                    nc.vector.tensor_copy(dst, src)
                    if ci == 1:
                        ones_dst = vt[b][0:tsz, tt, :].rearrange(
                            "p (h c) -> p h c", c=VW)[:, :, D:VW]
                        nc.vector.tensor_copy(
                            ones_dst,
                            ones_f[0:tsz, 0:1].to_broadcast([tsz, H, 1]))

                def scores_pair(b, p):
                    # transposed scores S^T[s, q] for heads (2p, 2p+1),
                    # exp'd into ex[(b,p)] tiles [ssz, 2, 320] bf16
                    mq, mk = p, NPAIR + p
                    ext = epool.tile([128, len(S_TILES), 2, 320], bf16,
                                     tag="ex", name="ext")
                    ex[(b, p)] = ext
                    for st, (s0, ssz) in enumerate(S_TILES):
                        last = (st == len(S_TILES) - 1)
                        w = 320 if last else 256
                        sc = psB.tile([128, 1024], f32, tag="sc", name="sc")
                        for i in (0, 1):
                            pof = 64 * i
                            nc.tensor.matmul(
                                sc[0:ssz, 512 * i:512 * i + 256],
                                qk[b][pof:pof + 64, mk, s0:s0 + ssz],
                                qk[b][pof:pof + 64, mq, T:N],
                                start=True, stop=True,
                                tile_position=(pof, 0))
                            if last:
                                nc.tensor.matmul(
                                    sc[0:T, 512 * i + 256:512 * i + 320],
                                    qk[b][pof:pof + 64, mk, 0:T],
                                    qk[b][pof:pof + 64, mq, 0:T],
                                    start=True, stop=True,
                                    tile_position=(pof, 0))
                        gap_in = bass.AP(
                            tensor=sc.tensor, offset=sc.offset,
                            ap=[sc.ap[0], [512, 2], [1, w]])
                        nc.scalar.activation(out=ext[0:ssz, st, :, 0:w],
                                             in_=gap_in[0:ssz],
                                             func=Exp, scale=SCALE)

                def attnv_group(b, ci, half):
                    # token-major O[q, 65] for 6 heads -> psO [qsz, 390]
                    q0, qsz, exoff = Q_CHUNKS[ci]
                    O = psO.tile([128, HH * VW], f32, tag="O", name="O")
                    for j in range(HH):
                        h = half * HH + j
                        p, i = divmod(h, 2)
                        if ci == 0:   # template queries: template keys only
                            nc.tensor.matmul(
                                O[0:qsz, j * VW:(j + 1) * VW],
                                ex[(b, p)][0:T, 2, i, 256:320],
                                vt[b][0:T, 0, h * VW:(h + 1) * VW],
                                start=True, stop=True)
                        else:         # search queries: all keys
                            for st, (s0, ssz) in enumerate(S_TILES):
                                nc.tensor.matmul(
                                    O[0:qsz, j * VW:(j + 1) * VW],
                                    ex[(b, p)][0:ssz, st, i,
                                               exoff:exoff + qsz],
                                    vt[b][0:ssz, st, h * VW:(h + 1) * VW],
                                    start=(st == 0),
                                    stop=(st == len(S_TILES) - 1))
                    ot = opool.tile([128, HH * VW], f32, tag="osb", name="ot")
                    osb[(b, ci, half)] = ot
                    nc.vector.tensor_copy(ot[0:qsz, :], O[0:qsz, :])

                def norm_half(b, ci, half):
                    # per-partition softmax normalize; write token-major
                    # bf16 attn rows [q, 6*64] for this half
                    q0, qsz, _ = Q_CHUNKS[ci]
                    if half == 0:
                        tok[(b, ci)] = tokpool.tile([128, C], bf16,
                                                    tag="tok", name="tok")
                    ot = osb[(b, ci, half)]
                    rec = rpool.tile([128, HH], f32, tag="rec", name="rec")
                    den = bass.AP(tensor=ot.tensor, offset=ot.offset + D,
                                  ap=[ot.ap[0], [VW, HH]])
                    nc.vector.reciprocal(out=rec[0:qsz, :], in_=den[0:qsz])
                    num = ot[0:qsz, :].rearrange(
                        "p (h c) -> p h c", c=VW)[:, :, 0:D]
                    recb = bass.AP(tensor=rec.tensor, offset=rec.offset,
                                   ap=[rec.ap[0], [1, HH], [0, D]])
                    dst = tok[(b, ci)][0:qsz, half * HH * D:
                                       (half + 1) * HH * D].rearrange(
                        "p (h c) -> p h c", c=D)
                    nc.vector.tensor_mul(dst, num, recb[0:qsz])

                def dmat_chunk(b, ci, half=None):
                    # DMA-XBAR transpose token-major -> feature-major.
                    # One fat call per chunk: out[p, ct, q] = in[q, ct*128+p]
                    # (3D out AP iterates the middle dim as the outer block
                    # index -- verified on hardware).  half=0/1 transposes
                    # just that half's columns (used by the last batch so
                    # half 0's transpose overlaps half 1's attention).
                    q0, qsz, _ = Q_CHUNKS[ci]
                    if b not in att:
                        att[b] = atpool.tile([128, NCT, N], bf16, name="att")
                    if half is None:
                        nc.sync.dma_start_transpose(
                            out=att[b][:, :, q0:q0 + qsz],
                            in_=tok[(b, ci)][0:qsz, :])
                    else:
                        nc.sync.dma_start_transpose(
                            out=att[b][:, half * 3:half * 3 + 3, q0:q0 + qsz],
                            in_=tok[(b, ci)][0:qsz,
                                             half * 384:(half + 1) * 384])

                def proj_chunk(b, m):
                    yp = psA.tile([128, N], f32, tag="yp", name="yp")
                    for ct in range(NCT):
                        nc.tensor.matmul(
                            yp[:, 0:N],
                            wp_sb[:, m, ct],
                            att[b][:, ct, :],
                            start=(ct == 0), stop=(ct == NCT - 1))
                    yt = ypool.tile([128, N], f32, tag="yt")
                    nc.scalar.activation(out=yt[:], in_=yp[:, 0:N],
                                         func=Identity,
                                         bias=pb_sb[:, m:m + 1])
                    nc.gpsimd.dma_start(
                        out=y_d[b, m * 128:(m + 1) * 128, :], in_=yt[:])

                def release(b):
                    for d_ in (xt, qk, vt, att):
                        d_.pop(b, None)
                    for p in range(NPAIR):
                        ex.pop((b, p), None)
                    for ci in range(3):
                        tok.pop((b, ci), None)
                        for half in (0, 1):
                            osb.pop((b, ci, half), None)

                # ---------- software-pipelined emission ----------
                # batch b's attention phase carries batch b+1's qkv+v
                # chunks (interleaved) and prefetches xt for b+2.
                def att_items(b):
                    last = (b == bpc - 1)
                    yield lambda: scores_pair(b, 0)
                    yield lambda: scores_pair(b, 1)
                    yield lambda: scores_pair(b, 2)
                    for ci in range(3):
                        yield lambda ci=ci: attnv_group(b, ci, 0)
                    for ci in range(3):
                        if last:
                            yield lambda ci=ci: (norm_half(b, ci, 0),
                                                 dmat_chunk(b, ci, half=0))
                        else:
                            yield lambda ci=ci: norm_half(b, ci, 0)
                    yield lambda: scores_pair(b, 3)
                    yield lambda: scores_pair(b, 4)
                    yield lambda: scores_pair(b, 5)
                    for ci in range(3):
                        yield lambda ci=ci: attnv_group(b, ci, 1)
                    for ci in range(3):
                        if last:
                            yield lambda ci=ci: (norm_half(b, ci, 1),
                                                 dmat_chunk(b, ci, half=1))
                        else:
                            yield lambda ci=ci: (norm_half(b, ci, 1),
                                                 dmat_chunk(b, ci))
                    for m in range(NCT):
                        yield lambda m=m: proj_chunk(b, m)

                def fill_items(b):
                    # next batch's qkv row-tiles interleaved with its v
                    # chunks (different PSUM rings, so adjacent fills never
                    # WAR-stall on the same accumulator).
                    for c in range(6):
                        yield lambda m=c: qkv_chunk(b, m)
                        yield lambda c=c: v_chunk(b, c)
                    for m in range(6, NQK):
                        yield lambda m=m: qkv_chunk(b, m)

                # prologue: batch 0/1 x loads (hoisted for reps==1) +
                # batch 0's qkv and v
                if 0 not in xt:
                    load_x(0)
                    if bpc > 1:
                        load_x(1)
                for it in fill_items(0):
                    it()

                for b in range(bpc):
                    if b + 2 < bpc:
                        load_x(b + 2)
                    fill = list(fill_items(b + 1)) if b + 1 < bpc else []
                    fi = 0
                    for k, it in enumerate(att_items(b)):
                        it()
                        if fi < len(fill):
                            fill[fi]()
                            fi += 1
                    while fi < len(fill):
                        fill[fi]()
                        fi += 1
                    release(b)

            if reps == 1:
                body()
            elif reps == 0:
                with tc.For_i(0, rv, 1) as _i:
                    body(_i)
            else:
                with tc.For_i(0, reps, 1) as _i:
                    body(_i)

    nc.compile()
    return nc


_NC_CACHE = {}


def _get_nc(bpc: int = BPC):
    if bpc not in _NC_CACHE:
        _NC_CACHE[bpc] = build_bass(bpc)
    return _NC_CACHE[bpc]


def make_in_maps(x1, x2, qkv_w, proj_w, proj_b, n_cores=N_CORES):
    import ml_dtypes
    bf = ml_dtypes.bfloat16

    x1 = np.asarray(x1, dtype=np.float32)
    x2 = np.asarray(x2, dtype=np.float32)
    qkv_w = np.asarray(qkv_w, dtype=np.float32)
    proj_w = np.asarray(proj_w, dtype=np.float32)
    proj_b = np.asarray(proj_b, dtype=np.float32)

    b = x1.shape[0]
    xt = np.empty((b, C, N), dtype=bf)
    xt[:, :, :T] = x1[:, :T, :].transpose(0, 2, 1).astype(bf)
    xt[:, :, T:] = x2[:, T:, :].transpose(0, 2, 1).astype(bf)
    # tile to [b, p, ct, t] so the per-batch DMA is fully contiguous
    xt = np.ascontiguousarray(
        xt.reshape(b, NCT, 128, N).transpose(0, 2, 1, 3))

    def tile_w(w):   # [C_in, M] -> [mt, p, ct, mm]
        ci, m = w.shape
        return np.ascontiguousarray(
            w.reshape(NCT, 128, m // 128, 128).transpose(2, 1, 0, 3)
        ).astype(bf)

    wqk = tile_w(qkv_w[:2 * C].T.astype(np.float32))
    wv = tile_w(qkv_w[2 * C:].T.astype(np.float32))
    wp = tile_w(proj_w.T.astype(np.float32))
    pb = np.ascontiguousarray(proj_b.reshape(NCT, 128))

    bpc = b // n_cores
    return [
        {"xt": xt[c * bpc:(c + 1) * bpc], "wqk": wqk, "wv": wv, "wp": wp,
         "pb": pb}
        for c in range(n_cores)
    ], bpc


def kernel(x1, x2, qkv_w, proj_w, proj_b):
    in_maps, bpc = make_in_maps(x1, x2, qkv_w, proj_w, proj_b)
    nc = _get_nc(bpc)
    res = run_bass_kernel_spmd(nc, in_maps, list(range(N_CORES)))
    yt = np.concatenate([res.results[c]["y"] for c in range(N_CORES)], axis=0)
    return np.ascontiguousarray(yt.transpose(0, 2, 1))
